# revision 1
# baseline (speedup 1.0000x reference)
"""Causal self-attention kernel for 8 Trainium2 NeuronCores.

Problem: y = CausalSelfAttention(x) with B=2, T=2048, C=2048, 16 heads,
head_dim=128, fp32 weights/activations.

Sharding (8 cores): core = (b, hg) with b in {0,1} (batch), hg in {0..3}
(head-group of 4 heads). Each core:
  phase 1: Q^T, K^T (as [hd, T]) and V (as [T, hd]) for its 4 heads from
           x[b]^T (host-transposed) and its W_attn column slices.
  phase 2: causal attention per head in transposed layout:
           S^T[k,q] = K Q^T (PE), E = exp(S^T/sqrt(hd)) (ACT, PSUM->SBUF),
           causal 0/1 mask on diagonal blocks (DVE),
           y^T[d,q] += V^T... via lhsT=V tile (PE, PSUM accum over k),
           denom[1,q] += ones^T E (PE), then y^T *= 1/denom broadcast.
  phase 3: partial c_proj: out[q,c] = sum_h y_h^T.T @ W_proj rows (PE),
           + b_proj (core hg==0 only; zeros elsewhere).
Host: out[b] = sum of the 4 head-group partials.

No collectives; one SPMD NEFF, per-core input data differs.
"""

import numpy as np

import concourse.bass as bass
import concourse.mybir as mybir
import concourse.tile as tile
from concourse.bass_utils import run_bass_kernel_spmd

B, T, C = 2, 2048, 2048
N_HEAD = 16
HD = 128
HPC = 4          # heads per core
HCOLS = HPC * HD  # 512 columns of W per core per projection
P = 128          # partitions
QT = 512         # q-tile (free dim) in attention
KT = 128         # k-tile in attention
NQT = T // QT    # 4
NCT = C // P     # 16 contraction tiles for projections
SCALE = 1.0 / np.sqrt(HD)

USE_FP32R = True
EXP_PAIR = False   # batch exp over two k-tiles (2-bank PSUM S tiles)

F32 = mybir.dt.float32
F32R = mybir.dt.float32r


# dtype for every tensor that feeds the PE (produced-as-fp32r satisfies the
# BIR verifier; fp32r matmuls run at 4x the rate of fp32 for free dim >= 256)
MMDT = F32R if USE_FP32R else F32


def build_nc(split_waits=True):
    nc = bass.Bass("TRN2", target_bir_lowering=False, debug=False)

    xT = nc.dram_tensor("xT", [C, T], MMDT, kind="ExternalInput").ap()
    wq = nc.dram_tensor("wq", [C, HCOLS], MMDT, kind="ExternalInput").ap()
    wk = nc.dram_tensor("wk", [C, HCOLS], MMDT, kind="ExternalInput").ap()
    wv = nc.dram_tensor("wv", [C, HCOLS], MMDT, kind="ExternalInput").ap()
    bq = nc.dram_tensor("bq", [P, HPC], F32, kind="ExternalInput").ap()
    bk = nc.dram_tensor("bk", [P, HPC], F32, kind="ExternalInput").ap()
    bv = nc.dram_tensor("bv", [P, HCOLS], F32, kind="ExternalInput").ap()
    wp = nc.dram_tensor("wp", [HCOLS, C], MMDT, kind="ExternalInput").ap()
    masks = nc.dram_tensor("masks", [QT // KT, P, QT], F32, kind="ExternalInput").ap()
    ones = nc.dram_tensor("ones", [P, P], MMDT, kind="ExternalInput").ap()
    out = nc.dram_tensor("out", [T, C], F32, kind="ExternalOutput").ap()

    with tile.TileContext(nc) as tc:
        _build(tc, xT, wq, wk, wv, bq, bk, bv, wp, masks, ones, out)
    if split_waits:
        _split_matmul_waits(nc)
    return nc


def _split_matmul_waits(nc):
    """Lowered instructions fit only ONE sync-wait command (walrus: 'Too many
    sync wait commands'; seen for self-loading fp32r Matmult and DMACopy).
    Move excess waits onto preceding same-engine EventSemaphore instructions
    (which hold 2 waits each)."""
    n = 0
    for f in nc.m.functions:
        for b in f.blocks:
            patched = []
            changed = False
            for inst in b.instructions:
                si = inst.sync_info
                if (
                    not isinstance(inst, mybir.InstEventSemaphore)
                    and si is not None
                    and len(si.on_wait) > 1
                ):
                    waits = list(si.on_wait)
                    extra, keep = waits[:-1], waits[-1:]
                    for ci in range(0, len(extra), 2):
                        n += 1
                        patched.append(
                            mybir.InstEventSemaphore(
                                name=f"{inst.name}-wsplit{ci}",
                                engine=inst.engine,
                                ins=[],
                                outs=[],
                                sync_info=mybir.SyncInfo(
                                    on_wait=extra[ci:ci + 2], on_update=[]
                                ),
                            )
                        )
                    si.on_wait = keep
                    changed = True
                patched.append(inst)
            if changed:
                b.instructions = patched
    return n


def _build(tc, xT, wq, wk, wv, bq, bk, bv, wp, masks, ones, out):
    from contextlib import ExitStack

    nc = tc.nc
    Exp = mybir.ActivationFunctionType.Exp
    Ident = mybir.ActivationFunctionType.Identity
    Copy = mybir.ActivationFunctionType.Copy

    with ExitStack() as root:
        # ---- SBUF residents ----------------------------------------------
        res_qkv = root.enter_context(tc.tile_pool(name="res_qkv", bufs=1))
        qt_sb = res_qkv.tile([P, HPC, T], MMDT, tag="qt_sb")   # Q^T per head
        kt_sb = res_qkv.tile([P, HPC, T], MMDT, tag="kt_sb")   # K^T per head
        v_sb = res_qkv.tile([P, NCT, HCOLS], MMDT, tag="v_sb")  # V [k, kt, (h d)]

        consts = root.enter_context(tc.tile_pool(name="consts", bufs=1))
        bq_sb = consts.tile([P, HPC], F32, tag="bq_sb")
        nc.gpsimd.dma_start(out=bq_sb[:, :], in_=bq)
        bk_sb = consts.tile([P, HPC], F32, tag="bk_sb")
        nc.gpsimd.dma_start(out=bk_sb[:, :], in_=bk)
        bv_sb = consts.tile([P, HCOLS], F32, tag="bv_sb")
        nc.gpsimd.dma_start(out=bv_sb[:, :], in_=bv)

        # ---- phase 1: QKV projections ------------------------------------
        with ExitStack() as ph1:
            wbig = ph1.enter_context(tc.tile_pool(name="wbig", bufs=2))
            xpool = ph1.enter_context(tc.tile_pool(name="xt", bufs=8))
            pj_psum = ph1.enter_context(
                tc.tile_pool(name="pj_psum", bufs=8, space="PSUM")
            )

            def load_w(w_dram, nm, order):
                slab = wbig.tile([P, NCT, HCOLS], MMDT, tag="wbig", name=nm)
                halves = []
                for i in order:
                    halves.append(nc.sync.dma_start(
                        out=slab[:, i * (NCT // 2):(i + 1) * (NCT // 2), :],
                        in_=w_dram.rearrange("(co ci) n -> ci co n", ci=P)[
                            :, i * (NCT // 2):(i + 1) * (NCT // 2), :],
                    ))
                return slab

            # pass A: Q and K together (one xT stream, 8 psum banks)
            PREF = 6

            def xt_dma(ci, tt, nm):
                t = xpool.tile([P, QT], MMDT, tag="xt", name=nm)
                eng = nc.sync if ci % 2 == 0 else nc.gpsimd
                eng.dma_start(
                    out=t[:, :],
                    in_=xT[ci * P:(ci + 1) * P, tt * QT:(tt + 1) * QT],
                )
                return t

            wq_sb = wbig.tile([P, NCT, HCOLS], MMDT, tag="wbig", name="wq_sb")
            wk_sb = wbig.tile([P, NCT, HCOLS], MMDT, tag="wbig", name="wk_sb")
            wqr = wq.rearrange("(co ci) n -> ci co n", ci=P)
            wkr = wk.rearrange("(co ci) n -> ci co n", ci=P)
            W_CHUNKS = [(0, 2), (2, 4), (4, 8), (8, 16)]

            def w_chunk(i):
                sl = slice(*W_CHUNKS[i])
                nc.sync.dma_start(out=wq_sb[:, sl, :], in_=wqr[:, sl, :])
                nc.gpsimd.dma_start(out=wk_sb[:, sl, :], in_=wkr[:, sl, :])

            w_chunk(0)
            pref = {(0, ci): xt_dma(ci, 0, f"xpre{ci}") for ci in range(2)}
            w_chunk(1)
            pref.update({(0, ci): xt_dma(ci, 0, f"xpre{ci}")
                         for ci in range(2, PREF)})
            for i in range(2, 4):
                w_chunk(i)
            for tt in range(NQT):
                psq = [pj_psum.tile([P, QT], F32, tag="pj", name=f"pq{i}")
                       for i in range(HPC)]
                psk = [pj_psum.tile([P, QT], F32, tag="pj", name=f"pk{i}")
                       for i in range(HPC)]
                for ci in range(NCT):
                    xt_t = pref.pop((tt, ci), None)
                    if xt_t is None:
                        xt_t = xt_dma(ci, tt, f"xa{tt}_{ci}")
                    for h in range(HPC):
                        nc.tensor.matmul(
                            psq[h][:, :],
                            (wq_sb[:, ci, h * HD:(h + 1) * HD]),
                            (xt_t[:, :]),
                            start=(ci == 0),
                            stop=(ci == NCT - 1),
                        )
                    for h in range(HPC):
                        nc.tensor.matmul(
                            psk[h][:, :],
                            (wk_sb[:, ci, h * HD:(h + 1) * HD]),
                            (xt_t[:, :]),
                            start=(ci == 0),
                            stop=(ci == NCT - 1),
                        )
                for h in range(HPC):
                    nc.scalar.activation(
                        qt_sb[:, h, tt * QT:(tt + 1) * QT], psq[h][:, :],
                        Ident, bias=bq_sb[:, h:h + 1],
                    )
                for h in range(HPC):
                    nc.scalar.activation(
                        kt_sb[:, h, tt * QT:(tt + 1) * QT], psk[h][:, :],
                        Ident, bias=bk_sb[:, h:h + 1],
                    )

            # pass B: V (second xT stream); prefetch its first tiles now
            for ci in range(PREF):
                pref[("b", 0, ci)] = xt_dma(ci, 0, f"xbpre{ci}")
            wv_sb = wbig.tile([P, NCT, HCOLS], MMDT, tag="wbig", name="wv_sb")
            wvr = wv.rearrange("(co ci) n -> ci co n", ci=P)
            for i in range(4):
                sl = slice(i * (NCT // 4), (i + 1) * (NCT // 4))
                (nc.sync if i % 2 == 0 else nc.gpsimd).dma_start(
                    out=wv_sb[:, sl, :], in_=wvr[:, sl, :])
            for tt in range(NQT):
                psv = [pj_psum.tile([P, HCOLS], F32, tag="pj", name=f"pv{i}")
                       for i in range(4)]
                for ci in range(NCT):
                    xt_t = pref.pop(("b", tt, ci), None)
                    if xt_t is None:
                        xt_t = xt_dma(ci, tt, f"xb{tt}_{ci}")
                    for ts in range(4):
                        nc.tensor.matmul(
                            psv[ts][:, :],
                            (xt_t[:, ts * P:(ts + 1) * P]),
                            (wv_sb[:, ci, :]),
                            start=(ci == 0),
                            stop=(ci == NCT - 1),
                        )
                for ts in range(4):
                    nc.vector.tensor_add(
                        v_sb[:, tt * 4 + ts, :], psv[ts][:, :], bv_sb[:, :]
                    )

        # ---- phase 2: causal attention, transposed layout ----------------
        ones_sb = consts.tile([P, P], MMDT, tag="ones_sb")
        nc.sync.dma_start(out=ones_sb[:, :], in_=ones)
        mask_sb = consts.tile([P, QT // KT, QT], F32, tag="mask_sb")
        nc.sync.dma_start(out=mask_sb[:, :, :], in_=masks.rearrange("j p q -> p j q"))
        # W_proj first half prefetches during attention.
        wppool = root.enter_context(tc.tile_pool(name="wppool", bufs=2))
        wp_r = wp.rearrange("(ht p) c -> p ht c", p=P)
        wp_sbs = [wppool.tile([P, HPC // 2, C], MMDT, tag="wp", name="wp0")]
        nc.sync.dma_start(out=wp_sbs[0][:, :, :], in_=wp_r[:, 0:2, :])

        res_yt = root.enter_context(tc.tile_pool(name="res_yt", bufs=1))
        yt_sb = res_yt.tile([P, HPC, T], MMDT, tag="yt_sb")   # y^T per head

        with ExitStack() as ph2:
            s_psum = ph2.enter_context(tc.tile_pool(name="s_psum", bufs=4, space="PSUM"))
            y_psum = ph2.enter_context(tc.tile_pool(name="y_psum", bufs=2, space="PSUM"))
            a_psum = ph2.enter_context(tc.tile_pool(name="a_psum", bufs=2, space="PSUM"))
            epool = ph2.enter_context(tc.tile_pool(name="epool", bufs=5))
            npool = ph2.enter_context(tc.tile_pool(name="npool", bufs=2))

            # software pipeline: the y/den matmuls for k-tile kt are emitted
            # after the NEXT S matmul, so the in-order PE queue never parks
            # right behind the exp it just requested.
            state = {}   # per (h, qt): psum tiles
            pending = None  # (h, qt, kt, e_sb)

            def emit_yden(h, qt, kt, e_sb):
                nkt = 4 * qt + 4
                y_ps, den_ps = state[(h, qt)]
                nc.tensor.matmul(
                    y_ps[:, :],
                    (v_sb[:, kt, h * HD:(h + 1) * HD]),
                    (e_sb[:, :]),
                    start=(kt == 0),
                    stop=(kt == nkt - 1),
                )
                nc.tensor.matmul(
                    den_ps[:, :],
                    (ones_sb[:, :]),
                    (e_sb[:, :]),
                    start=(kt == 0),
                    stop=(kt == nkt - 1),
                )
                if kt == nkt - 1:
                    # den_ps rows all identical (ones-matrix matmul):
                    # reciprocal gives the broadcast 1/den directly
                    rbc = npool.tile([P, QT], F32, tag="rbc", name=f"rbc{h}_{qt}")
                    nc.vector.reciprocal(rbc[:, :], den_ps[:, :])
                    nc.vector.tensor_mul(
                        yt_sb[:, h, qt * QT:(qt + 1) * QT], y_ps[:, :], rbc[:, :]
                    )
                    del state[(h, qt)]

            for h in range(HPC):
                for qt in range(NQT):
                    nkt = 4 * qt + 4
                    state[(h, qt)] = (
                        y_psum.tile([P, QT], F32, tag="y", name=f"y{h}_{qt}"),
                        a_psum.tile([P, QT], F32, tag="den", name=f"den{h}_{qt}"),
                    )
                    for kt in range(nkt):
                        s_ps = s_psum.tile([P, QT], F32, tag="s")
                        nc.tensor.matmul(
                            s_ps[:, :],
                            (kt_sb[:, h, kt * KT:(kt + 1) * KT]),
                            (qt_sb[:, h, qt * QT:(qt + 1) * QT]),
                            start=True,
                            stop=True,
                        )
                        if pending is not None:
                            emit_yden(*pending)
                        e_sb = epool.tile([P, QT], MMDT, tag="e")
                        nc.scalar.activation(
                            e_sb[:, :], s_ps[:, :], Exp, scale=float(SCALE)
                        )
                        j = kt - 4 * qt
                        if j >= 0:
                            nc.vector.tensor_mul(
                                e_sb[:, :], e_sb[:, :], mask_sb[:, j, :]
                            )
                        pending = (h, qt, kt, e_sb)
            emit_yden(*pending)

        # ---- phase 3: partial c_proj (b_proj added host-side) ------------
        wp_sbs.append(wppool.tile([P, HPC // 2, C], MMDT, tag="wp", name="wp1"))
        nc.sync.dma_start(out=wp_sbs[1][:, :, :], in_=wp_r[:, 2:4, :])

        with ExitStack() as ph3:
            cp_psum = ph3.enter_context(tc.tile_pool(name="cp_psum", bufs=4, space="PSUM"))
            opool = ph3.enter_context(tc.tile_pool(name="opool", bufs=2))

            for qi in range(T // P):  # 16 q-tiles of 128 rows
                o_sb = opool.tile([P, C], F32, tag="o_sb")
                for ct in range(C // QT):  # 4 column tiles of 512
                    cp = cp_psum.tile([P, QT], F32, tag="cp")
                    for h in range(HPC):
                        nc.tensor.matmul(
                            cp[:, :],
                            (yt_sb[:, h, qi * P:(qi + 1) * P]),
                            (wp_sbs[h // 2][:, h % 2, ct * QT:(ct + 1) * QT]),
                            start=(h == 0),
                            stop=(h == HPC - 1),
                        )
                    nc.vector.tensor_copy(
                        o_sb[:, ct * QT:(ct + 1) * QT], cp[:, :]
                    )
                nc.sync.dma_start(out=out[qi * P:(qi + 1) * P, :], in_=o_sb[:, :])


def make_core_inputs(x, W_attn, b_attn, W_proj, b_proj):
    """Host-side shard/prep. Returns list of 8 input dicts (np.float32)."""
    x = np.asarray(x, dtype=np.float32)
    W_attn = np.asarray(W_attn, dtype=np.float32)
    b_attn = np.asarray(b_attn, dtype=np.float32)
    W_proj = np.asarray(W_proj, dtype=np.float32)
    b_proj = np.asarray(b_proj, dtype=np.float32)

    njt = QT // KT
    mask = np.zeros((njt, P, QT), dtype=np.float32)
    for j in range(njt):
        k_idx = j * KT + np.arange(P)[:, None]
        q_idx = np.arange(QT)[None, :]
        mask[j] = (k_idx <= q_idx).astype(np.float32)

    in_maps = []
    for core in range(8):
        b, hg = divmod(core, 4)
        cs = slice(HCOLS * hg, HCOLS * hg + HCOLS)
        in_maps.append(
            {
                "xT": np.ascontiguousarray(x[b].T),
                "wq": np.ascontiguousarray(W_attn[:, 0 * C:1 * C][:, cs]),
                "wk": np.ascontiguousarray(W_attn[:, 1 * C:2 * C][:, cs]),
                "wv": np.ascontiguousarray(W_attn[:, 2 * C:3 * C][:, cs]),
                "bq": np.ascontiguousarray(
                    b_attn[0 * C:1 * C][cs].reshape(HPC, HD).T
                ),
                "bk": np.ascontiguousarray(
                    b_attn[1 * C:2 * C][cs].reshape(HPC, HD).T
                ),
                "bv": np.ascontiguousarray(
                    np.broadcast_to(b_attn[2 * C:3 * C][cs], (P, HCOLS))
                ),
                "wp": np.ascontiguousarray(W_proj[cs, :]),
                "masks": mask,
                "ones": np.ones((P, P), dtype=np.float32),
            }
        )
    return in_maps


_NC_CACHE = {}


def get_nc(split_waits=True):
    key = ("nc", split_waits)
    if key not in _NC_CACHE:
        _NC_CACHE[key] = build_nc(split_waits)
    return _NC_CACHE[key]


def kernel(x, W_attn, b_attn, W_proj, b_proj):
    in_maps = make_core_inputs(x, W_attn, b_attn, W_proj, b_proj)
    nc = get_nc()
    res = run_bass_kernel_spmd(nc, in_maps, core_ids=list(range(8)))
    parts = [r["out"] for r in res.results]
    y = np.empty((B, T, C), dtype=np.float32)
    bpf = np.asarray(b_proj, dtype=np.float32)
    for b in range(B):
        y[b] = parts[4 * b] + parts[4 * b + 1] + parts[4 * b + 2] + parts[4 * b + 3]
        y[b] += bpf
    return y


if __name__ == "__main__":
    rng = np.random.default_rng(0)
    x = rng.standard_normal((B, T, C), dtype=np.float32)
    W_attn = rng.standard_normal((C, 3 * C), dtype=np.float32) / np.sqrt(C)
    b_attn = rng.standard_normal(3 * C).astype(np.float32) * 0.02
    W_proj = rng.standard_normal((C, C), dtype=np.float32) / np.sqrt(C)
    b_proj = rng.standard_normal(C).astype(np.float32) * 0.02
    y = kernel(x, W_attn, b_attn, W_proj, b_proj)
    print(y.shape, y.dtype, float(np.abs(y).mean()))

